# revision 1
# baseline (speedup 1.0000x reference)
"""Trainium2 Bass kernel for nn_IterativeClassifier (B=65536, D=512, E=64, C=10, T=40).

Strategy (pure data parallel over 8 cores, batch-sharded):
  All activations live TRANSPOSED on-chip: [E, batch] with batch on the free dim.
  The z-state is never materialized. Using relu positive-homogeneity and the
  de-scaled substitution  h^_t := 0.9^-t * h_t, the whole 40-step recurrence
  becomes a pair of persistent PSUM accumulators per batch tile:
    HA_t = 0.9^-t * (W1f@F + W1z@z_t)    (PSUM, matmul-accumulated)
    L    = logits accumulator            (PSUM, matmul-accumulated)
  Per step and batch-tile only THREE small matmuls (K=64,M<=64, quadrant-tiled
  across the PE array) and ONE PSUM->SBUF evacuation (relu+bias, alternating
  ScalarE/VectorE) are needed:
    HA += (0.1/0.9 * W1z@W2) @ h^_{t-1}      (mm_a)
    L  += (0.1 * CE@W2) @ h^_{t-1}           (mm_L)
    HA += (0.1 * 0.9^-t * W1f) @ F           (mm_b, per-step prescaled weights)
    h^_t = relu(HA + beta_t)                 (evac)
  Finally logits = 0.9^39 * L + biasL.

  Per core: 16 batch tiles of 512 columns, paired onto 128 partitions
  (tile A on partitions 0:64, tile B on 64:128), 2 chunks of 4 pairs
  (PSUM: 4 HA banks + 4 L banks = all 8 banks per chunk).
"""

import numpy as np

import concourse.bass as bass
import concourse.bacc as bacc
import concourse.mybir as mybir
import concourse.tile as tile
from concourse.bass_utils import run_bass_kernel_spmd

F32 = mybir.dt.float32
AF = mybir.ActivationFunctionType
ALU = mybir.AluOpType

NCORES = 8
B, D, E, C, T = 65536, 512, 64, 10, 40
DEC, LR = 0.9, 0.1
NT = 512                      # batch columns per tile
BSH = B // NCORES             # 8192 batch rows per core
TILES = BSH // NT             # 16
PAIRS = TILES // 2            # 8
CHUNK_PAIRS = 4
CHUNKS = PAIRS // CHUNK_PAIRS # 2


def _host_prep(x, z0, W_feat, b_feat, W1, b1, W2, b2, class_emb):
    """All host-side numpy preprocessing: transposed/prescaled weights + shards."""
    f4 = np.float32
    W1f = W1[:, :E].astype(f4)
    W1z = W1[:, E:2 * E].astype(f4)
    w1t = W1[:, 2 * E].astype(f4)

    def dup(a):  # stack a [64,m] lhsT onto both partition halves -> [128,m]
        return np.concatenate([a, a], axis=0).astype(f4)

    Gp = (LR / DEC) * (W1z @ W2)                       # [64,64]
    CL = LR * (class_emb @ W2)                         # [10,64]
    CEi = DEC * class_emb                              # [10,64]
    wf_blocks = [W1f.T] + [(LR * DEC ** (-t)) * W1f.T for t in range(1, T)]
    wf = dup(np.concatenate(wf_blocks, axis=1))        # [128, 40*64]
    wg = dup(Gp.T)                                     # [128, 64]
    w1zbd = np.zeros((128, 128), f4)                   # blockdiag: one full-bank init
    w1zbd[0:E, 0:E] = W1z.T
    w1zbd[E:128, E:128] = W1z.T
    clp = np.zeros((E, E), f4); clp[:, :C] = CL.T
    cl = dup(clp)                                      # [128, 64]
    ceibd = np.zeros((128, 128), f4)                   # anti-blockdiag L init
    ceibd[0:E, E:E + C] = CEi.T
    ceibd[E:128, 0:C] = CEi.T
    wfeat = W_feat.T.reshape(4, 128, E).transpose(1, 0, 2).reshape(128, 4 * E).astype(f4)
    # wfeat[p, 64k+m] = W_feat.T[128k+p, m] -> slice [:, 64k:64k+64] is chunk k
    beta = np.stack([
        DEC ** (-t) * (b1 + (t / T) * w1t + (1 - DEC ** t) * (W1z @ b2) + W1f @ b_feat)
        for t in range(T)
    ]).T.astype(f4)                                    # [64, 40]
    beta = np.concatenate([beta, beta], axis=0)        # [128, 40]
    biasl = np.zeros((128, 1), f4)
    bl = ((1 - DEC ** T) * (class_emb @ b2)).astype(f4)
    biasl[0:C, 0] = bl
    biasl[64:64 + C, 0] = bl

    # x -> per-core per-tile [128, 4*NT] blocks:  x_dev[c,i,p,k*NT+n] = x[c*BSH+i*NT+n, 128k+p]
    xr = x.astype(f4).reshape(NCORES, TILES, NT, 4, 128).transpose(0, 1, 4, 3, 2)
    x_dev = np.ascontiguousarray(xr.reshape(NCORES, TILES, 128, 4 * NT))
    # z0 -> per-core per-pair [128, NT]: rows 0:64 = tile 2p, rows 64:128 = tile 2p+1
    zr = z0.astype(f4).reshape(NCORES, PAIRS, 2, NT, E).transpose(0, 1, 2, 4, 3)
    z0_dev = np.ascontiguousarray(zr.reshape(NCORES, PAIRS, 128, NT))

    consts = np.concatenate(
        [wfeat, wg, wf, w1zbd, cl, ceibd, beta, biasl], axis=1).astype(f4)
    return {"consts_d": consts}, x_dev, z0_dev


def build(n_tiles=TILES, t_steps=T):
    """Build the Bass module. Returns nc."""
    n_pairs = n_tiles // 2
    chunk_pairs = min(CHUNK_PAIRS, n_pairs)
    nc = bacc.Bacc("TRN2", target_bir_lowering=False, debug=False)

    x_d = nc.dram_tensor("x_d", [n_tiles, 128, 4 * NT], F32, kind="ExternalInput").ap()
    z0_d = nc.dram_tensor("z0_d", [n_pairs, 128, NT], F32, kind="ExternalInput").ap()
    NCONST = 4 * E + E + T * E + 128 + E + 128 + T + 1
    consts_d = nc.dram_tensor("consts_d", [128, NCONST], F32, kind="ExternalInput").ap()
    out_d = nc.dram_tensor("out_d", [n_tiles, C, NT], F32, kind="ExternalOutput").ap()

    scale_l = float(DEC ** (t_steps - 1))

    with tile.TileContext(nc) as tc:
        with (
            tc.sbuf_pool(name="consts", bufs=1) as cpool,
            tc.sbuf_pool(name="xt", bufs=8) as xpool,
            tc.sbuf_pool(name="ff", bufs=chunk_pairs) as ffpool,
            tc.sbuf_pool(name="hh", bufs=2 * chunk_pairs) as hhpool,
            tc.sbuf_pool(name="z0s", bufs=2) as zpool,
            tc.sbuf_pool(name="ll", bufs=2) as llpool,
            tc.psum_pool(name="ha", bufs=chunk_pairs) as hapool,
            tc.psum_pool(name="lb", bufs=chunk_pairs) as lbpool,
        ):
            const_sb = cpool.tile([128, NCONST], F32)
            nc.sync.dma_start(const_sb, consts_d)
            o = 0
            def _sl(n):
                nonlocal o
                v = const_sb[:, o:o + n]; o += n; return v
            wfeat_sb = _sl(4 * E); wg_sb = _sl(E); wf_sb = _sl(T * E)
            w1zbd_sb = _sl(128); cl_sb = _sl(E); ceibd_sb = _sl(128)
            beta_sb = _sl(T); biasl_sb = _sl(1)

            LO, HI = slice(0, 64), slice(64, 128)

            def evac(pi, dst, src, bias_ap):
                # h^ = relu(src + beta): alternate engines to split the load
                if pi % 2 == 0:
                    nc.scalar.activation(dst, src, AF.Relu, bias=bias_ap, scale=1.0)
                else:
                    nc.vector.tensor_scalar(dst, src, bias_ap, 0.0, ALU.add, ALU.max)

            for chunk in range((n_pairs + chunk_pairs - 1) // chunk_pairs):
                pairs = range(chunk * chunk_pairs,
                              min((chunk + 1) * chunk_pairs, n_pairs))
                HA, LB, FF, HH = {}, {}, {}, {}
                # ---- feature + init phase ----
                for p in pairs:
                    fp = hapool.tile([128, NT], F32, tag="ha", name=f"fp{p}")
                    for ab in range(2):  # ab=0 -> tile A=2p (F at HI), ab=1 -> B (F at LO)
                        dst = fp[HI] if ab == 0 else fp[LO]
                        for k in range(4):
                            xt = xpool.tile([128, NT], F32, tag="xt",
                                            name=f"xt{p}_{ab}_{k}")
                            nc.gpsimd.dma_start(xt, x_d[2 * p + ab, :, NT * k:NT * (k + 1)])
                            nc.tensor.matmul(dst, wfeat_sb[:, E * k:E * (k + 1)],
                                             xt, start=(k == 0), stop=(k == 3), skip_group_check=True)
                    ff = ffpool.tile([128, NT], F32, tag="ff", name=f"ff{p}")
                    nc.scalar.activation(ff, fp, AF.Copy, bias=0.0, scale=1.0)
                    FF[p] = ff

                    z0t = zpool.tile([128, NT], F32, tag="z0s", name=f"z0t{p}")
                    nc.gpsimd.dma_start(z0t, z0_d[p])
                    ha = hapool.tile([128, NT], F32, tag="ha", name=f"ha{p}")
                    lb = lbpool.tile([128, NT], F32, tag="lb", name=f"lb{p}")
                    HA[p], LB[p] = ha, lb
                    # HA_0 = W1z@z0 (one full-bank matmul opens the only group)
                    nc.tensor.matmul(ha, w1zbd_sb, z0t, start=True, stop=False, skip_group_check=True)
                    nc.tensor.matmul(ha[LO], wf_sb[HI, 0:E], ff[HI], start=False, stop=False, skip_group_check=True)
                    nc.tensor.matmul(ha[HI], wf_sb[LO, 0:E], ff[LO], start=False, stop=True, skip_group_check=True)
                    # L_init = (0.9*CE)@z0  (anti-blockdiag: A -> L[HI], B -> L[LO])
                    nc.tensor.matmul(lb, ceibd_sb, z0t, start=True, stop=False, skip_group_check=True)
                    hh = hhpool.tile([128, NT], F32, tag="hh", name=f"hh{p}_0")
                    evac(p, hh, ha, beta_sb[:, 0:1])
                    HH[p] = hh

                # ---- 39 recurrence steps ----
                for t in range(1, t_steps):
                    last = t == t_steps - 1
                    for p in pairs:  # mm_a on diagonal quadrants (G' stationary)
                        nc.tensor.matmul(HA[p][LO], wg_sb[LO], HH[p][LO],
                                         start=False, stop=False, skip_group_check=True)
                        nc.tensor.matmul(HA[p][HI], wg_sb[HI], HH[p][HI],
                                         start=False, stop=False, skip_group_check=True)
                    for p in pairs:  # mm_L on anti-diagonal quadrants
                        nc.tensor.matmul(LB[p][HI], cl_sb[LO], HH[p][LO],
                                         start=False, stop=False, skip_group_check=True)
                        nc.tensor.matmul(LB[p][LO], cl_sb[HI], HH[p][HI],
                                         start=False, stop=False, skip_group_check=True)
                    for p in pairs:  # mm_b on anti-diagonal quadrants
                        nc.tensor.matmul(HA[p][LO], wf_sb[HI, E * t:E * (t + 1)],
                                         FF[p][HI], start=False, stop=False, skip_group_check=True)
                        nc.tensor.matmul(HA[p][HI], wf_sb[LO, E * t:E * (t + 1)],
                                         FF[p][LO], start=False, stop=True, skip_group_check=True)
                    for p in pairs:
                        hh = hhpool.tile([128, NT], F32, tag="hh", name=f"hh{p}_{t}")
                        evac(p, hh, HA[p], beta_sb[:, t:t + 1])
                        HH[p] = hh

                # ---- final: last mm_L, logits evac + store ----
                for p in pairs:
                    nc.tensor.matmul(LB[p][HI], cl_sb[LO], HH[p][LO],
                                     start=False, stop=False, skip_group_check=True)
                    nc.tensor.matmul(LB[p][LO], cl_sb[HI], HH[p][HI],
                                     start=False, stop=True, skip_group_check=True)
                    ll = llpool.tile([128, NT], F32, tag="ll", name=f"ll{p}")
                    nc.scalar.activation(ll, LB[p], AF.Identity,
                                         bias=biasl_sb[:, 0:1], scale=scale_l)
                    nc.sync.dma_start(out_d[2 * p], ll[64:64 + C, :])
                    nc.sync.dma_start(out_d[2 * p + 1], ll[0:C, :])
    nc.compile()
    return nc


_BUILT = {}


def _get_nc(n_tiles=TILES, t_steps=T):
    key = (n_tiles, t_steps)
    if key not in _BUILT:
        _BUILT[key] = build(n_tiles, t_steps)
    return _BUILT[key]


def kernel(x, z0, W_feat, b_feat, W1, b1, W2, b2, class_emb, T_steps, **run_kw):
    x = np.asarray(x); z0 = np.asarray(z0)
    assert int(T_steps) == T
    const, x_dev, z0_dev = _host_prep(
        np.asarray(x), np.asarray(z0), np.asarray(W_feat), np.asarray(b_feat),
        np.asarray(W1), np.asarray(b1), np.asarray(W2), np.asarray(b2),
        np.asarray(class_emb))
    nc = _get_nc()
    in_maps = []
    for c in range(NCORES):
        m = dict(const)
        m["x_d"] = x_dev[c]
        m["z0_d"] = z0_dev[c]
        in_maps.append(m)
    res = run_bass_kernel_spmd(nc, in_maps, core_ids=list(range(NCORES)), **run_kw)
    outs = [r["out_d"] for r in res.results]  # each [TILES, C, NT]
    # out[c][i, cc, n] -> logits[c*BSH + i*NT + n, cc]
    stacked = np.stack(outs)                       # [8, 16, 10, 512]
    logits = stacked.transpose(0, 1, 3, 2).reshape(B, C)
    if run_kw:
        kernel.last_result = res
    return np.ascontiguousarray(logits.astype(np.float32))



# revision 2
# speedup vs baseline: 2.4844x; 2.4844x over previous
"""Trainium2 Bass kernel for nn_IterativeClassifier (B=65536, D=512, E=64, C=10, T=40).

Strategy (pure data parallel over 8 cores, batch-sharded):
  All activations live TRANSPOSED on-chip: [E, batch] with batch on the free dim.
  The z-state is never materialized. Using relu positive-homogeneity and the
  de-scaled substitution  h^_t := 0.9^-t * h_t, the whole 40-step recurrence
  becomes a pair of persistent PSUM accumulators per batch tile:
    HA_t = 0.9^-t * (W1f@F + W1z@z_t)    (PSUM, matmul-accumulated)
    L    = logits accumulator            (PSUM, matmul-accumulated)
  Per step and batch-tile only THREE small matmuls (K=64,M<=64, quadrant-tiled
  across the PE array) and ONE PSUM->SBUF evacuation (relu+bias, alternating
  ScalarE/VectorE) are needed:
    HA += (0.1/0.9 * W1z@W2) @ h^_{t-1}      (mm_a)
    L  += (0.1 * CE@W2) @ h^_{t-1}           (mm_L)
    HA += (0.1 * 0.9^-t * W1f) @ F           (mm_b, per-step prescaled weights)
    h^_t = relu(HA + beta_t)                 (evac)
  Finally logits = 0.9^39 * L + biasL.

  All matmul operands are bf16 (4x PE throughput vs fp32; fp32 PSUM
  accumulation; rel-err ~2e-3 vs 2e-2 budget). mm_L quadrants alternate by
  pair parity so all four 64x64 PE quadrant streams carry an equal load
  (6 matmuls/quadrant/step/chunk instead of 4/4/8/8).

  Per core: 16 batch tiles of 512 columns, paired onto 128 partitions
  (tile A on partitions 0:64, tile B on 64:128), 2 chunks of 4 pairs
  (PSUM: 4 HA banks + 4 L banks = all 8 banks per chunk).
"""

import ml_dtypes
import numpy as np

import concourse.bass as bass
import concourse.bacc as bacc
import concourse.mybir as mybir
import concourse.tile as tile
from concourse.bass_utils import run_bass_kernel_spmd

F32 = mybir.dt.float32
BF16 = mybir.dt.bfloat16
AF = mybir.ActivationFunctionType
ALU = mybir.AluOpType

NCORES = 8
B, D, E, C, T = 65536, 512, 64, 10, 40
DEC, LR = 0.9, 0.1
NT = 512                      # batch columns per tile
BSH = B // NCORES             # 8192 batch rows per core
TILES = BSH // NT             # 16
PAIRS = TILES // 2            # 8
CHUNK_PAIRS = 4
CHUNKS = PAIRS // CHUNK_PAIRS # 2

NBF = ml_dtypes.bfloat16


def _host_prep(x, z0, W_feat, b_feat, W1, b1, W2, b2, class_emb):
    """All host-side numpy preprocessing: transposed/prescaled weights + shards."""
    f4 = np.float32
    W1f = W1[:, :E].astype(f4)
    W1z = W1[:, E:2 * E].astype(f4)
    w1t = W1[:, 2 * E].astype(f4)

    def dup(a):  # stack a [64,m] lhsT onto both partition halves -> [128,m]
        return np.concatenate([a, a], axis=0).astype(f4)

    Gp = (LR / DEC) * (W1z @ W2)                       # [64,64]
    CL = LR * (class_emb @ W2)                         # [10,64]
    CEi = DEC * class_emb                              # [10,64]
    wf_blocks = [W1f.T] + [(LR * DEC ** (-t)) * W1f.T for t in range(1, T)]
    wf = dup(np.concatenate(wf_blocks, axis=1))        # [128, 40*64]
    wg = dup(Gp.T)                                     # [128, 64]
    w1zbd = np.zeros((128, 128), f4)                   # blockdiag: one full-bank init
    w1zbd[0:E, 0:E] = W1z.T
    w1zbd[E:128, E:128] = W1z.T
    clp = np.zeros((E, E), f4); clp[:, :C] = CL.T
    cl = dup(clp)                                      # [128, 64]
    ceanti = np.zeros((128, 128), f4)                  # anti-blockdiag L init (odd pairs)
    ceanti[0:E, E:E + C] = CEi.T
    ceanti[E:128, 0:C] = CEi.T
    ceblk = np.zeros((128, 128), f4)                   # blockdiag L init (even pairs)
    ceblk[0:E, 0:C] = CEi.T
    ceblk[E:128, E:E + C] = CEi.T
    wfeat = W_feat.T.reshape(4, 128, E).transpose(1, 0, 2).reshape(128, 4 * E).astype(f4)
    # wfeat[p, 64k+m] = W_feat.T[128k+p, m] -> slice [:, 64k:64k+64] is chunk k
    beta = np.stack([
        DEC ** (-t) * (b1 + (t / T) * w1t + (1 - DEC ** t) * (W1z @ b2) + W1f @ b_feat)
        for t in range(T)
    ]).T.astype(f4)                                    # [64, 40]
    beta = np.concatenate([beta, beta], axis=0)        # [128, 40]
    biasl = np.zeros((128, 1), f4)
    bl = ((1 - DEC ** T) * (class_emb @ b2)).astype(f4)
    biasl[0:C, 0] = bl
    biasl[64:64 + C, 0] = bl

    # x -> per-core per-tile [128, 4*NT] blocks:  x_dev[c,i,p,k*NT+n] = x[c*BSH+i*NT+n, 128k+p]
    xr = x.astype(f4).reshape(NCORES, TILES, NT, 4, 128).transpose(0, 1, 4, 3, 2)
    x_dev = np.ascontiguousarray(xr.reshape(NCORES, TILES, 128, 4 * NT)).astype(NBF)
    # z0 -> per-core per-pair [128, NT]: rows 0:64 = tile 2p, rows 64:128 = tile 2p+1
    zr = z0.astype(f4).reshape(NCORES, PAIRS, 2, NT, E).transpose(0, 1, 2, 4, 3)
    z0_dev = np.ascontiguousarray(zr.reshape(NCORES, PAIRS, 128, NT)).astype(NBF)

    consts = np.concatenate(
        [wfeat, wg, wf, w1zbd, cl, ceanti, ceblk], axis=1).astype(NBF)
    consts32 = np.concatenate([beta, biasl], axis=1).astype(f4)
    return {"consts_d": consts, "consts32_d": consts32}, x_dev, z0_dev


def build(n_tiles=TILES, t_steps=T):
    """Build the Bass module. Returns nc."""
    n_pairs = n_tiles // 2
    chunk_pairs = min(CHUNK_PAIRS, n_pairs)
    nc = bacc.Bacc("TRN2", target_bir_lowering=False, debug=False)

    x_d = nc.dram_tensor("x_d", [n_tiles, 128, 4 * NT], BF16, kind="ExternalInput").ap()
    z0_d = nc.dram_tensor("z0_d", [n_pairs, 128, NT], BF16, kind="ExternalInput").ap()
    NCONST = 4 * E + E + T * E + 128 + E + 128 + 128
    consts_d = nc.dram_tensor("consts_d", [128, NCONST], BF16, kind="ExternalInput").ap()
    consts32_d = nc.dram_tensor("consts32_d", [128, T + 1], F32, kind="ExternalInput").ap()
    out_d = nc.dram_tensor("out_d", [n_tiles, C, NT], F32, kind="ExternalOutput").ap()

    scale_l = float(DEC ** (t_steps - 1))

    with tile.TileContext(nc) as tc:
        with (
            tc.sbuf_pool(name="consts", bufs=1) as cpool,
            tc.sbuf_pool(name="consts32", bufs=1) as cpool32,
            tc.sbuf_pool(name="xt", bufs=8) as xpool,
            tc.sbuf_pool(name="ff", bufs=chunk_pairs) as ffpool,
            tc.sbuf_pool(name="hh", bufs=2 * chunk_pairs) as hhpool,
            tc.sbuf_pool(name="z0s", bufs=2) as zpool,
            tc.sbuf_pool(name="ll", bufs=2) as llpool,
            tc.psum_pool(name="ha", bufs=chunk_pairs) as hapool,
            tc.psum_pool(name="lb", bufs=chunk_pairs) as lbpool,
        ):
            const_sb = cpool.tile([128, NCONST], BF16)
            nc.sync.dma_start(const_sb, consts_d)
            const32_sb = cpool32.tile([128, T + 1], F32)
            nc.sync.dma_start(const32_sb, consts32_d)
            o = 0
            def _sl(n):
                nonlocal o
                v = const_sb[:, o:o + n]; o += n; return v
            wfeat_sb = _sl(4 * E); wg_sb = _sl(E); wf_sb = _sl(T * E)
            w1zbd_sb = _sl(128); cl_sb = _sl(E)
            ceanti_sb = _sl(128); ceblk_sb = _sl(128)
            beta_sb = const32_sb[:, 0:T]
            biasl_sb = const32_sb[:, T:T + 1]

            LO, HI = slice(0, 64), slice(64, 128)

            def evac(pi, dst, src, bias_ap):
                # h^ = relu(src + beta): alternate engines to split the load
                if pi % 2 == 0:
                    nc.scalar.activation(dst, src, AF.Relu, bias=bias_ap, scale=1.0)
                else:
                    nc.vector.tensor_scalar(dst, src, bias_ap, 0.0, ALU.add, ALU.max)

            def mm_L(p, lb, hh, start=False, stop=False):
                # quadrant-balanced: even pairs use diagonal quadrants,
                # odd pairs anti-diagonal. L_A lives at LB rows 0:C (even) /
                # 64:64+C (odd); L_B at the other half.
                if p % 2 == 0:
                    nc.tensor.matmul(lb[LO], cl_sb[LO], hh[LO],
                                     start=start, stop=stop, skip_group_check=True)
                    nc.tensor.matmul(lb[HI], cl_sb[HI], hh[HI],
                                     start=start, stop=stop, skip_group_check=True)
                else:
                    nc.tensor.matmul(lb[HI], cl_sb[LO], hh[LO],
                                     start=start, stop=stop, skip_group_check=True)
                    nc.tensor.matmul(lb[LO], cl_sb[HI], hh[HI],
                                     start=start, stop=stop, skip_group_check=True)

            for chunk in range((n_pairs + chunk_pairs - 1) // chunk_pairs):
                pairs = range(chunk * chunk_pairs,
                              min((chunk + 1) * chunk_pairs, n_pairs))
                HA, LB, FF, HH = {}, {}, {}, {}
                # ---- feature + init phase ----
                for p in pairs:
                    fp = hapool.tile([128, NT], F32, tag="ha", name=f"fp{p}")
                    for ab in range(2):  # ab=0 -> tile A=2p (F at HI), ab=1 -> B (F at LO)
                        dst = fp[HI] if ab == 0 else fp[LO]
                        for k in range(4):
                            xt = xpool.tile([128, NT], BF16, tag="xt",
                                            name=f"xt{p}_{ab}_{k}")
                            nc.gpsimd.dma_start(xt, x_d[2 * p + ab, :, NT * k:NT * (k + 1)])
                            nc.tensor.matmul(dst, wfeat_sb[:, E * k:E * (k + 1)],
                                             xt, start=(k == 0), stop=(k == 3), skip_group_check=True)
                    ff = ffpool.tile([128, NT], BF16, tag="ff", name=f"ff{p}")
                    nc.scalar.activation(ff, fp, AF.Copy, bias=0.0, scale=1.0)
                    FF[p] = ff

                    z0t = zpool.tile([128, NT], BF16, tag="z0s", name=f"z0t{p}")
                    nc.gpsimd.dma_start(z0t, z0_d[p])
                    ha = hapool.tile([128, NT], F32, tag="ha", name=f"ha{p}")
                    lb = lbpool.tile([128, NT], F32, tag="lb", name=f"lb{p}")
                    HA[p], LB[p] = ha, lb
                    # HA_0 = W1z@z0 (one full-bank matmul opens the only group)
                    nc.tensor.matmul(ha, w1zbd_sb, z0t, start=True, stop=False, skip_group_check=True)
                    nc.tensor.matmul(ha[LO], wf_sb[HI, 0:E], ff[HI], start=False, stop=False, skip_group_check=True)
                    nc.tensor.matmul(ha[HI], wf_sb[LO, 0:E], ff[LO], start=False, stop=True, skip_group_check=True)
                    # L_init = (0.9*CE)@z0 (diag/anti-diag placement by parity)
                    ce_sb = ceblk_sb if p % 2 == 0 else ceanti_sb
                    nc.tensor.matmul(lb, ce_sb, z0t, start=True, stop=False, skip_group_check=True)
                    hh = hhpool.tile([128, NT], BF16, tag="hh", name=f"hh{p}_0")
                    evac(p, hh, ha, beta_sb[:, 0:1])
                    HH[p] = hh

                # ---- 39 recurrence steps ----
                for t in range(1, t_steps):
                    for p in pairs:  # mm_a on diagonal quadrants (G' stationary)
                        nc.tensor.matmul(HA[p][LO], wg_sb[LO], HH[p][LO],
                                         start=False, stop=False, skip_group_check=True)
                        nc.tensor.matmul(HA[p][HI], wg_sb[HI], HH[p][HI],
                                         start=False, stop=False, skip_group_check=True)
                    for p in pairs:  # mm_L, quadrant parity by pair
                        mm_L(p, LB[p], HH[p])
                    for p in pairs:  # mm_b on anti-diagonal quadrants
                        nc.tensor.matmul(HA[p][LO], wf_sb[HI, E * t:E * (t + 1)],
                                         FF[p][HI], start=False, stop=False, skip_group_check=True)
                        nc.tensor.matmul(HA[p][HI], wf_sb[LO, E * t:E * (t + 1)],
                                         FF[p][LO], start=False, stop=True, skip_group_check=True)
                    for p in pairs:
                        hh = hhpool.tile([128, NT], BF16, tag="hh", name=f"hh{p}_{t}")
                        evac(p, hh, HA[p], beta_sb[:, t:t + 1])
                        HH[p] = hh

                # ---- final: last mm_L, logits evac + store ----
                for p in pairs:
                    mm_L(p, LB[p], HH[p], stop=True)
                    ll = llpool.tile([128, NT], F32, tag="ll", name=f"ll{p}")
                    nc.scalar.activation(ll, LB[p], AF.Identity,
                                         bias=biasl_sb[:, 0:1], scale=scale_l)
                    a_sl, b_sl = ((slice(0, C), slice(64, 64 + C)) if p % 2 == 0
                                  else (slice(64, 64 + C), slice(0, C)))
                    nc.sync.dma_start(out_d[2 * p], ll[a_sl, :])
                    nc.sync.dma_start(out_d[2 * p + 1], ll[b_sl, :])
    nc.compile()
    return nc


_BUILT = {}


def _get_nc(n_tiles=TILES, t_steps=T):
    key = (n_tiles, t_steps)
    if key not in _BUILT:
        _BUILT[key] = build(n_tiles, t_steps)
    return _BUILT[key]


def kernel(x, z0, W_feat, b_feat, W1, b1, W2, b2, class_emb, T_steps, **run_kw):
    x = np.asarray(x); z0 = np.asarray(z0)
    assert int(T_steps) == T
    const, x_dev, z0_dev = _host_prep(
        np.asarray(x), np.asarray(z0), np.asarray(W_feat), np.asarray(b_feat),
        np.asarray(W1), np.asarray(b1), np.asarray(W2), np.asarray(b2),
        np.asarray(class_emb))
    nc = _get_nc()
    in_maps = []
    for c in range(NCORES):
        m = dict(const)
        m["x_d"] = x_dev[c]
        m["z0_d"] = z0_dev[c]
        in_maps.append(m)
    res = run_bass_kernel_spmd(nc, in_maps, core_ids=list(range(NCORES)), **run_kw)
    outs = [r["out_d"] for r in res.results]  # each [TILES, C, NT]
    # out[c][i, cc, n] -> logits[c*BSH + i*NT + n, cc]
    stacked = np.stack(outs)                       # [8, 16, 10, 512]
    logits = stacked.transpose(0, 1, 3, 2).reshape(B, C)
    if run_kw:
        kernel.last_result = res
    return np.ascontiguousarray(logits.astype(np.float32))
